# revision 30
# baseline (speedup 1.0000x reference)
"""MoE (brute-force reference) kernel for 8 TRN2 NeuronCores.

Strategy: expert-parallel, 2 experts per core in two capacity slots.
Host routes tokens by gate_idx (top-k dedup), assigns the 8 largest
experts to slot 0 (capacity C0=256) and the 8 smallest to slot 1
(C1=238), transposes so the device sees xt[slot] = X_e.T [D, C]. Each
core runs
  hT[m] = gelu(sum_k w1T[k,m].T @ xT[k] + b1)   then
  yT[m] = sum_k w2T[k,m].T @ hT[k]
All matmul operands are fp16 (same PE rate as bf16); accumulation is
fp32 in PSUM. b1 is fused into the gelu activation; b2 and the
gate_score combine happen on host in exact fp32. Tokens beyond a
slot's capacity (~2% under the seed-0 routing) fall back to an exact
host path.

Schedule notes (from NTFF traces):
- C<=256 keeps each PSUM chain within half a 2KB bank, giving 16
  logical banks: GEMM1 group0 -> ps0-7, group1 -> ps8-15, GEMM2 ->
  ps0-7 again. The scalar engine's gelu evictions (~368ns each) then
  never gate the PE at group boundaries.
- Both HWDGE issue chains are used: scalar gets exactly 6 w1a slabs
  (so its gelu table load + activations follow early), sync carries
  everything else in strict consumption order. DMA issue slots are
  ~0.65us and the global DMA-sem pool only allows ~11 outstanding
  transfers, so issue order == need order matters.
- A vector-memset-fed dummy-matmul warm-up keeps the PE HAM activity
  window busy from the instant engines start (~6.5us) so the 1.2GHz
  cold clock ramps to 2.4GHz just as the real stream begins.
- GEMM2 is k-outer (m-inner over 8 banks) for k-tiles 0..11 so weight
  chunks are consumed as they arrive, then a per-m tail (k12-15 +
  eviction) streams the y DMAs out before the kernel end.
"""

import numpy as np

import concourse.bacc as bacc
import concourse.mybir as mybir
from concourse import tile
from concourse.bass_utils import run_bass_kernel_spmd

E, D, H, TOPK, T = 16, 1024, 2048, 2, 2048
NCORES = 8
EPC = E // NCORES          # experts (slots) per core
CAPS = (256, 238)          # slot capacities; seed-0 overflow -> host
KD, KH, MD = D // 128, H // 128, D // 128  # 8, 16, 8

_F16 = np.float16
_CACHE: dict = {}


def _build(reps: int = 1):
    dt = mybir.dt.float16
    f32 = mybir.dt.float32
    nc = bacc.Bacc("TRN2", target_bir_lowering=False, debug=False,
                   num_devices=NCORES)
    # xt is packed partition-major on host ([128, KD*C], line = C*2B*KD
    # contiguous per partition) so its DMA moves 2KB+ lines, not 512B.
    # w1 is pre-split into column halves so every slab/chunk DMA reads
    # fully contiguous DRAM (the [D, H] layout made slab reads 2KB-of-4KB
    # strided, halving effective HBM read bandwidth). w2 chunks read full
    # rows of [H, D] and are already contiguous. yt is partition-major so
    # the y writes are contiguous too.
    xts = [nc.dram_tensor(f"xt{e}", [128, KD * CAPS[e]], dt,
                          kind="ExternalInput") for e in range(EPC)]
    w1a_d = nc.dram_tensor("w1a", [EPC, D, H // 2], dt, kind="ExternalInput")
    w1b_d = nc.dram_tensor("w1b", [EPC, D, H // 2], dt, kind="ExternalInput")
    w2t = nc.dram_tensor("w2t", [EPC, H, D], dt, kind="ExternalInput")
    b1 = nc.dram_tensor("b1", [EPC, 128, KH], f32, kind="ExternalInput")
    yts = [nc.dram_tensor(f"yt{e}", [128, MD * CAPS[e]], dt,
                          kind="ExternalOutput") for e in range(EPC)]

    gelu = mybir.ActivationFunctionType.Gelu_apprx_tanh
    MGRP = 8
    HH = H // 2

    with tile.TileContext(nc) as tc:
        with (
            tc.tile_pool(name="xtp", bufs=1) as xtp,
            tc.tile_pool(name="w1p", bufs=1) as w1p,
            tc.tile_pool(name="w2p", bufs=1) as w2p,
            tc.tile_pool(name="htp", bufs=1) as htp,
            tc.tile_pool(name="yp", bufs=1) as yp,
            tc.tile_pool(name="bp", bufs=1) as bp,
            tc.tile_pool(name="ps", bufs=1, space="PSUM") as psp,
        ):
            # PSUM: 8 physical banks, each [128, 512] f32, allocated once.
            # C<=256 keeps every accumulation chain within half a bank, so
            # column slices give 16 independent chains (subtile deps).
            banks = [psp.tile([128, 512], f32, name=f"bank{i}", tag=f"ps{i}")
                     for i in range(MGRP)]

            # ---- warm-up (see module docstring)
            zt = bp.tile([128, 128], dt, name="warmz", tag="warmz")
            nc.vector.memset(zt[:], 0.0)
            for _ in range(32):
                nc.tensor.matmul(banks[7][:, :128], zt[:], zt[:],
                                 start=True, stop=True)

            # ---- DMA plumbing
            def w_dma(eng, pool, pfx, dram, e, k0, nk, col0, ncol):
                tl = pool.tile([128, nk * ncol], dt, name=f"{pfx}_{e}_{k0}",
                               tag=f"{pfx}{e}_{k0}")
                eng.dma_start(
                    out=tl[:].rearrange("p (k m) -> p k m", k=nk),
                    in_=dram.ap()[e, k0 * 128:(k0 + nk) * 128,
                                  col0:col0 + ncol]
                        .rearrange("(k p) m -> p k m", p=128))
                return [tl[:, j * ncol:(j + 1) * ncol] for j in range(nk)]

            def xt_dma(eng, e, k0, nk):
                C = CAPS[e]
                tl = xtp.tile([128, nk * C], dt, name=f"xt_{e}_{k0}",
                              tag=f"xt{e}_{k0}")
                eng.dma_start(out=tl[:],
                              in_=xts[e].ap()[:, k0 * C:(k0 + nk) * C])
                return [tl[:, j * C:(j + 1) * C] for j in range(nk)]

            # --- slot 0 startup, interleaved on both chains in need order.
            # Scalar gets only 4 slabs so its gelu table load + activations
            # are never pushed late by DMA-sem ring stalls.
            xv0 = [None] * KD
            w1a0 = [None] * KD

            def xt0_dma(eng, k0, nk):
                xv0[k0:k0 + nk] = xt_dma(eng, 0, k0, nk)

            def w1a_dma(eng, k):
                w1a0[k:k + 1] = w_dma(eng, w1p, "w1a", w1a_d, 0, k, 1, 0, HH)

            b1s0 = bp.tile([128, KH], f32, name="b1s0", tag="b1s0")
            xt0_dma(nc.sync, 0, 2)
            w1a_dma(nc.scalar, 0)
            xt0_dma(nc.sync, 2, 2)
            w1a_dma(nc.scalar, 1)
            w1a_dma(nc.sync, 2)
            w1a_dma(nc.scalar, 3)
            xt0_dma(nc.sync, 4, 2)
            w1a_dma(nc.scalar, 4)
            w1a_dma(nc.sync, 5)
            w1a_dma(nc.sync, 6)
            xt0_dma(nc.sync, 6, 2)
            w1a_dma(nc.sync, 7)
            nc.scalar.dma_start(out=b1s0[:], in_=b1.ap()[0])
            w1b0 = []
            w1b0 += w_dma(nc.scalar, w1p, "w1b", w1b_d, 0, 0, 2, 0, HH)
            w1b0 += w_dma(nc.scalar, w1p, "w1b", w1b_d, 0, 2, 2, 0, HH)
            # scalar chain ends here: gelu table load + activations follow.
            w1b0 += w_dma(nc.sync, w1p, "w1b", w1b_d, 0, 4, 2, 0, HH)
            w1b0 += w_dma(nc.sync, w1p, "w1b", w1b_d, 0, 6, 2, 0, HH)
            w2s0 = []
            for c in range(4):
                w2s0 += w_dma(nc.sync, w2p, "w2s", w2t, 0, 4 * c, 4, 0, D)

            # --- slot 1 transfers: coarse chunks, all on sync
            xv1 = xt_dma(nc.sync, 1, 0, KD)
            w1a1 = (w_dma(nc.sync, w1p, "w1A", w1a_d, 1, 0, 4, 0, HH) +
                    w_dma(nc.sync, w1p, "w1A", w1a_d, 1, 4, 4, 0, HH))
            b1s1 = bp.tile([128, KH], f32, name="b1s1", tag="b1s1")
            nc.sync.dma_start(out=b1s1[:], in_=b1.ap()[1])
            w1b1 = (w_dma(nc.sync, w1p, "w1B", w1b_d, 1, 0, 4, 0, HH) +
                    w_dma(nc.sync, w1p, "w1B", w1b_d, 1, 4, 4, 0, HH))
            w2s1 = []
            for c in range(4):
                w2s1 += w_dma(nc.sync, w2p, "w2s", w2t, 1, 4 * c, 4, 0, D)

            # ---- compute
            for e in range(EPC):
                C = CAPS[e]
                xv = xv0 if e == 0 else xv1
                w1h = [w1a0 if e == 0 else w1a1, w1b0 if e == 0 else w1b1]
                w2s = w2s0 if e == 0 else w2s1
                b1s = b1s0 if e == 0 else b1s1

                # GEMM1: group 0 -> bank halves [0:C], group 1 -> [256:256+C];
                # k-outer so group0 is paced by w1a slab arrival at startup.
                hts = [htp.tile([128, C], dt, name=f"ht{e}_{m}",
                                tag=f"ht{e}_{m}") for m in range(KH)]
                for g in range(2):
                    wsrc = w1h[g]
                    off = 256 * g
                    pss = [banks[i][:, off:off + C] for i in range(MGRP)]
                    for k in range(KD - 2):
                        for i in range(MGRP):
                            nc.tensor.matmul(
                                pss[i],
                                wsrc[k][:, i * 128:(i + 1) * 128],
                                xv[k],
                                start=(k == 0), stop=False)
                    # fused final two k-rounds: per-m (k6, k7, act) so the
                    # scalar activations start ~2us before the group ends
                    # and never gate the next group's PSUM banks.
                    for i, m in enumerate(range(g * MGRP, (g + 1) * MGRP)):
                        for k in (KD - 2, KD - 1):
                            nc.tensor.matmul(
                                pss[i],
                                wsrc[k][:, i * 128:(i + 1) * 128],
                                xv[k],
                                start=False, stop=(k == KD - 1))
                        nc.scalar.activation(
                            hts[m][:], pss[i], gelu,
                            bias=b1s[:, m:m + 1])

                # GEMM2 (bank halves [0:C], freed by group0's acts): k-outer
                # for k 0..11 (chunk-paced), then per-m tail (k12-15 +
                # eviction) so y DMAs stream before kernel end. For the last
                # slot the evictions ARE the kernel tail: y DMAs batch as
                # [m0-3, m4-6, m7] so the final transfer after the last
                # matmul is one small contiguous write.
                pss = [banks[m][:, 0:C] for m in range(MD)]
                for k in range(12):
                    for m in range(MD):
                        nc.tensor.matmul(
                            pss[m],
                            w2s[k][:, m * 128:(m + 1) * 128],
                            hts[k][:],
                            start=(k == 0), stop=False)
                last = e == EPC - 1
                yo = yp.tile([128, MD * C], dt, name=f"yo{e}", tag=f"yo{e}")
                for m in range(MD):
                    for k in range(12, KH):
                        nc.tensor.matmul(
                            pss[m],
                            w2s[k][:, m * 128:(m + 1) * 128],
                            hts[k][:],
                            start=False, stop=(k == KH - 1))
                    nc.vector.tensor_copy(
                        out=yo[:, m * C:(m + 1) * C], in_=pss[m])
                    if not last:
                        if m == 3 or m == MD - 1:
                            lo = 0 if m == 3 else 4 * C
                            nc.gpsimd.dma_start(
                                out=yts[e].ap()[:, lo:(m + 1) * C],
                                in_=yo[:, lo:(m + 1) * C])
                    elif m == 3:
                        nc.sync.dma_start(
                            out=yts[e].ap()[:, 0:4 * C],
                            in_=yo[:, 0:4 * C])
                    elif m == 6:
                        # scalar (the other HWDGE) so this issue doesn't
                        # serialize ahead of m7's final DMA on sync.
                        nc.scalar.dma_start(
                            out=yts[e].ap()[:, 4 * C:7 * C],
                            in_=yo[:, 4 * C:7 * C])
                    elif m == MD - 1:
                        nc.sync.dma_start(
                            out=yts[e].ap()[:, m * C:(m + 1) * C],
                            in_=yo[:, m * C:(m + 1) * C])
    nc.compile()
    return nc


def _get_nc(reps: int = 1):
    if reps not in _CACHE:
        _CACHE[reps] = _build(reps)
    return _CACHE[reps]


def _route(gate_idx, gate_score):
    """Dedup routing + slot assignment. Returns per-expert
    (tokens, weights, overflow_tokens, overflow_weights, core, slot)."""
    g = np.asarray(gate_idx).astype(np.int64)
    sc = np.asarray(gate_score, dtype=np.float32)
    toks_all, wts_all, counts = [], [], []
    for e in range(E):
        m0, m1 = g[:, 0] == e, g[:, 1] == e
        toks = np.flatnonzero(m0 | m1)
        toks_all.append(toks)
        wts_all.append((sc[:, 0] * m0 + sc[:, 1] * m1)[toks])
        counts.append(len(toks))
    order = np.argsort(-np.asarray(counts), kind="stable")
    out = [None] * E
    for rank, e in enumerate(order):
        slot = 0 if rank < NCORES else 1
        core = rank if rank < NCORES else rank - NCORES
        cap = CAPS[slot]
        toks, wts = toks_all[e], wts_all[e]
        out[e] = (toks[:cap], wts[:cap], toks[cap:], wts[cap:], core, slot)
    return out


def kernel(inp, gate_idx, gate_score, w1, b1, w2, b2):
    inp = np.asarray(inp, dtype=np.float32)
    gate_idx = np.asarray(gate_idx)
    gate_score = np.asarray(gate_score, dtype=np.float32)
    w1 = np.asarray(w1, dtype=np.float32)
    b1 = np.asarray(b1, dtype=np.float32)
    w2 = np.asarray(w2, dtype=np.float32)
    b2 = np.asarray(b2, dtype=np.float32)

    routes = _route(gate_idx, gate_score)

    w1t_all = w1.transpose(0, 2, 1).astype(_F16)  # [E, D, H]
    w2t_all = np.ascontiguousarray(w2.transpose(0, 2, 1)).astype(_F16)
    b1r = np.ascontiguousarray(
        b1.reshape(E, KH, 128).transpose(0, 2, 1))  # [E, 128, KH]

    in_maps = [
        {"w1a": np.zeros((EPC, D, H // 2), _F16),
         "w1b": np.zeros((EPC, D, H // 2), _F16),
         "w2t": np.zeros((EPC, H, D), _F16),
         "b1": np.zeros((EPC, 128, KH), np.float32),
         "xt0": np.zeros((128, KD * CAPS[0]), _F16),
         "xt1": np.zeros((128, KD * CAPS[1]), _F16)}
        for _ in range(NCORES)
    ]
    for e in range(E):
        toks, wts, otoks, owts, core, slot = routes[e]
        im = in_maps[core]
        im["w1a"][slot] = w1t_all[e][:, :H // 2]
        im["w1b"][slot] = w1t_all[e][:, H // 2:]
        im["w2t"][slot] = w2t_all[e]
        im["b1"][slot] = b1r[e]
        if len(toks):
            # pack partition-major: xt[p, k*C + c] = inp[toks[c], k*128 + p]
            xv = im[f"xt{slot}"].reshape(128, KD, CAPS[slot])
            xv[:, :, :len(toks)] = (
                inp[toks].T.astype(_F16)
                .reshape(KD, 128, len(toks)).transpose(1, 0, 2))

    nc = _get_nc()
    res = run_bass_kernel_spmd(nc, in_maps, list(range(NCORES)))

    # Host combine: weight each expert's output columns by the (summed)
    # gate score and accumulate per token; add the b2 term (folded out of
    # the device kernel). Tokens are unique within an expert, so the
    # fancy-indexed += is safe.
    out = np.einsum("tk,tkd->td", gate_score,
                    b2[np.asarray(gate_idx).astype(np.int64)])
    out = np.ascontiguousarray(out, dtype=np.float32)
    for e in range(E):
        toks, wts, otoks, owts, core, slot = routes[e]
        if len(toks):
            cap = CAPS[slot]
            # unpack partition-major yt [128, MD*C] -> [D, C]
            y = (res.results[core][f"yt{slot}"]
                 .reshape(128, MD, cap).transpose(1, 0, 2)
                 .reshape(D, cap)[:, :len(toks)].T)
            out[toks] += wts[:, None] * y.astype(np.float32)
        if len(otoks):  # exact host fallback for capacity overflow
            hh = inp[otoks] @ w1[e].T + b1[e]
            hh = 0.5 * hh * (1.0 + np.tanh(
                np.sqrt(2.0 / np.pi) * (hh + 0.044715 * hh ** 3)))
            out[otoks] += owts[:, None] * (hh @ w2[e].T)
    return out


# revision 31
# speedup vs baseline: 1.0108x; 1.0108x over previous
"""MoE (brute-force reference) kernel for 8 TRN2 NeuronCores.

Strategy: expert-parallel, 2 experts per core in two capacity slots.
Host routes tokens by gate_idx (top-k dedup), assigns the 8 largest
experts to slot 0 (capacity C0=256) and the 8 smallest to slot 1
(C1=238), transposes so the device sees xt[slot] = X_e.T [D, C]. Each
core runs
  hT[m] = gelu(sum_k w1T[k,m].T @ xT[k] + b1)   then
  yT[m] = sum_k w2T[k,m].T @ hT[k]
All matmul operands are fp16 (same PE rate as bf16); accumulation is
fp32 in PSUM. b1 is fused into the gelu activation; b2 and the
gate_score combine happen on host in exact fp32. Tokens beyond a
slot's capacity (~2% under the seed-0 routing) fall back to an exact
host path.

Schedule notes (from NTFF traces):
- C<=256 keeps each PSUM chain within half a 2KB bank, giving 16
  logical banks: GEMM1 group0 -> ps0-7, group1 -> ps8-15, GEMM2 ->
  ps0-7 again. The scalar engine's gelu evictions (~368ns each) then
  never gate the PE at group boundaries.
- Both HWDGE issue chains are used: scalar gets exactly 6 w1a slabs
  (so its gelu table load + activations follow early), sync carries
  everything else in strict consumption order. DMA issue slots are
  ~0.65us and the global DMA-sem pool only allows ~11 outstanding
  transfers, so issue order == need order matters.
- A vector-memset-fed dummy-matmul warm-up keeps the PE HAM activity
  window busy from the instant engines start (~6.5us) so the 1.2GHz
  cold clock ramps to 2.4GHz just as the real stream begins.
- GEMM2 is k-outer (m-inner over 8 banks) for k-tiles 0..11 so weight
  chunks are consumed as they arrive, then a per-m tail (k12-15 +
  eviction) streams the y DMAs out before the kernel end.
"""

import numpy as np

import concourse.bacc as bacc
import concourse.mybir as mybir
from concourse import tile
from concourse.bass_utils import run_bass_kernel_spmd

E, D, H, TOPK, T = 16, 1024, 2048, 2, 2048
NCORES = 8
EPC = E // NCORES          # experts (slots) per core
CAPS = (256, 238)          # slot capacities; seed-0 overflow -> host
KD, KH, MD = D // 128, H // 128, D // 128  # 8, 16, 8

_F16 = np.float16
_CACHE: dict = {}


def _build(reps: int = 1):
    dt = mybir.dt.float16
    f32 = mybir.dt.float32
    nc = bacc.Bacc("TRN2", target_bir_lowering=False, debug=False,
                   num_devices=NCORES)
    # xt is packed partition-major on host ([128, KD*C], line = C*2B*KD
    # contiguous per partition) so its DMA moves 2KB+ lines, not 512B.
    # w1 is pre-split into column halves so every slab/chunk DMA reads
    # fully contiguous DRAM (the [D, H] layout made slab reads 2KB-of-4KB
    # strided, halving effective HBM read bandwidth). w2 chunks read full
    # rows of [H, D] and are already contiguous. yt is partition-major so
    # the y writes are contiguous too.
    xts = [nc.dram_tensor(f"xt{e}", [128, KD * CAPS[e]], dt,
                          kind="ExternalInput") for e in range(EPC)]
    w1a_d = nc.dram_tensor("w1a", [EPC, D, H // 2], dt, kind="ExternalInput")
    w1b_d = nc.dram_tensor("w1b", [EPC, D, H // 2], dt, kind="ExternalInput")
    w2t = nc.dram_tensor("w2t", [EPC, H, D], dt, kind="ExternalInput")
    b1 = nc.dram_tensor("b1", [EPC, 128, KH], f32, kind="ExternalInput")
    yts = [nc.dram_tensor(f"yt{e}", [128, MD * CAPS[e]], dt,
                          kind="ExternalOutput") for e in range(EPC)]

    gelu = mybir.ActivationFunctionType.Gelu_apprx_tanh
    MGRP = 8
    HH = H // 2

    with tile.TileContext(nc) as tc:
        with (
            tc.tile_pool(name="xtp", bufs=1) as xtp,
            tc.tile_pool(name="w1p", bufs=1) as w1p,
            tc.tile_pool(name="w2p", bufs=1) as w2p,
            tc.tile_pool(name="htp", bufs=1) as htp,
            tc.tile_pool(name="yp", bufs=1) as yp,
            tc.tile_pool(name="bp", bufs=1) as bp,
            tc.tile_pool(name="ps", bufs=1, space="PSUM") as psp,
        ):
            # PSUM: 8 physical banks, each [128, 512] f32, allocated once.
            # C<=256 keeps every accumulation chain within half a bank, so
            # column slices give 16 independent chains (subtile deps).
            banks = [psp.tile([128, 512], f32, name=f"bank{i}", tag=f"ps{i}")
                     for i in range(MGRP)]

            # ---- warm-up (see module docstring)
            zt = bp.tile([128, 128], dt, name="warmz", tag="warmz")
            nc.vector.memset(zt[:], 0.0)
            for _ in range(32):
                nc.tensor.matmul(banks[7][:, :128], zt[:], zt[:],
                                 start=True, stop=True)

            # ---- DMA plumbing
            def w_dma(eng, pool, pfx, dram, e, k0, nk, col0, ncol):
                tl = pool.tile([128, nk * ncol], dt, name=f"{pfx}_{e}_{k0}",
                               tag=f"{pfx}{e}_{k0}")
                eng.dma_start(
                    out=tl[:].rearrange("p (k m) -> p k m", k=nk),
                    in_=dram.ap()[e, k0 * 128:(k0 + nk) * 128,
                                  col0:col0 + ncol]
                        .rearrange("(k p) m -> p k m", p=128))
                return [tl[:, j * ncol:(j + 1) * ncol] for j in range(nk)]

            def xt_dma(eng, e, k0, nk):
                C = CAPS[e]
                tl = xtp.tile([128, nk * C], dt, name=f"xt_{e}_{k0}",
                              tag=f"xt{e}_{k0}")
                eng.dma_start(out=tl[:],
                              in_=xts[e].ap()[:, k0 * C:(k0 + nk) * C])
                return [tl[:, j * C:(j + 1) * C] for j in range(nk)]

            # --- slot 0 startup, interleaved on both chains in need order.
            # Scalar gets only 4 slabs so its gelu table load + activations
            # are never pushed late by DMA-sem ring stalls.
            xv0 = [None] * KD
            w1a0 = [None] * KD

            def xt0_dma(eng, k0, nk):
                xv0[k0:k0 + nk] = xt_dma(eng, 0, k0, nk)

            def w1a_dma(eng, k):
                w1a0[k:k + 1] = w_dma(eng, w1p, "w1a", w1a_d, 0, k, 1, 0, HH)

            b1s0 = bp.tile([128, KH], f32, name="b1s0", tag="b1s0")
            xt0_dma(nc.sync, 0, 2)
            w1a_dma(nc.scalar, 0)
            xt0_dma(nc.sync, 2, 2)
            w1a_dma(nc.scalar, 1)
            w1a_dma(nc.sync, 2)
            w1a_dma(nc.scalar, 3)
            xt0_dma(nc.sync, 4, 2)
            w1a_dma(nc.scalar, 4)
            w1a_dma(nc.sync, 5)
            w1a_dma(nc.sync, 6)
            xt0_dma(nc.sync, 6, 2)
            w1a_dma(nc.sync, 7)
            nc.scalar.dma_start(out=b1s0[:], in_=b1.ap()[0])
            # scalar chain ends here: gelu table load + activations follow.
            # w1b: first k-slabs fine-grained so group 1's early k-rounds
            # start on first arrival instead of waiting a 2-slab chunk.
            w1b0 = []
            for k0, nk in ((0, 1), (1, 1), (2, 1), (3, 1), (4, 2), (6, 2)):
                w1b0 += w_dma(nc.sync, w1p, "w1b", w1b_d, 0, k0, nk, 0, HH)
            w2s0 = []
            for c in range(4):
                w2s0 += w_dma(nc.sync, w2p, "w2s", w2t, 0, 4 * c, 4, 0, D)

            # --- slot 1 transfers: coarse chunks, all on sync
            xv1 = xt_dma(nc.sync, 1, 0, KD)
            w1a1 = (w_dma(nc.sync, w1p, "w1A", w1a_d, 1, 0, 4, 0, HH) +
                    w_dma(nc.sync, w1p, "w1A", w1a_d, 1, 4, 4, 0, HH))
            b1s1 = bp.tile([128, KH], f32, name="b1s1", tag="b1s1")
            nc.sync.dma_start(out=b1s1[:], in_=b1.ap()[1])
            w1b1 = (w_dma(nc.sync, w1p, "w1B", w1b_d, 1, 0, 4, 0, HH) +
                    w_dma(nc.sync, w1p, "w1B", w1b_d, 1, 4, 4, 0, HH))
            w2s1 = []
            for c in range(4):
                w2s1 += w_dma(nc.sync, w2p, "w2s", w2t, 1, 4 * c, 4, 0, D)

            # ---- compute
            for e in range(EPC):
                C = CAPS[e]
                xv = xv0 if e == 0 else xv1
                w1h = [w1a0 if e == 0 else w1a1, w1b0 if e == 0 else w1b1]
                w2s = w2s0 if e == 0 else w2s1
                b1s = b1s0 if e == 0 else b1s1

                # GEMM1: group 0 -> bank halves [0:C], group 1 -> [256:256+C];
                # k-outer so group0 is paced by w1a slab arrival at startup.
                hts = [htp.tile([128, C], dt, name=f"ht{e}_{m}",
                                tag=f"ht{e}_{m}") for m in range(KH)]
                for g in range(2):
                    wsrc = w1h[g]
                    off = 256 * g
                    pss = [banks[i][:, off:off + C] for i in range(MGRP)]
                    for k in range(KD - 2):
                        for i in range(MGRP):
                            nc.tensor.matmul(
                                pss[i],
                                wsrc[k][:, i * 128:(i + 1) * 128],
                                xv[k],
                                start=(k == 0), stop=False)
                    # fused final two k-rounds: per-m (k6, k7, act) so the
                    # scalar activations start ~2us before the group ends
                    # and never gate the next group's PSUM banks.
                    for i, m in enumerate(range(g * MGRP, (g + 1) * MGRP)):
                        for k in (KD - 2, KD - 1):
                            nc.tensor.matmul(
                                pss[i],
                                wsrc[k][:, i * 128:(i + 1) * 128],
                                xv[k],
                                start=False, stop=(k == KD - 1))
                        nc.scalar.activation(
                            hts[m][:], pss[i], gelu,
                            bias=b1s[:, m:m + 1])

                # GEMM2 (bank halves [0:C], freed by group0's acts): k-outer
                # for k 0..11 (chunk-paced), then per-m tail (k12-15 +
                # eviction) so y DMAs stream before kernel end. For the last
                # slot the evictions ARE the kernel tail: y DMAs batch as
                # [m0-3, m4-6, m7] so the final transfer after the last
                # matmul is one small contiguous write.
                pss = [banks[m][:, 0:C] for m in range(MD)]
                for k in range(12):
                    for m in range(MD):
                        nc.tensor.matmul(
                            pss[m],
                            w2s[k][:, m * 128:(m + 1) * 128],
                            hts[k][:],
                            start=(k == 0), stop=False)
                last = e == EPC - 1
                yo = yp.tile([128, MD * C], dt, name=f"yo{e}", tag=f"yo{e}")
                for m in range(MD):
                    for k in range(12, KH):
                        nc.tensor.matmul(
                            pss[m],
                            w2s[k][:, m * 128:(m + 1) * 128],
                            hts[k][:],
                            start=False, stop=(k == KH - 1))
                    nc.vector.tensor_copy(
                        out=yo[:, m * C:(m + 1) * C], in_=pss[m])
                    if not last:
                        if m == 3 or m == MD - 1:
                            lo = 0 if m == 3 else 4 * C
                            nc.gpsimd.dma_start(
                                out=yts[e].ap()[:, lo:(m + 1) * C],
                                in_=yo[:, lo:(m + 1) * C])
                    elif m == 3:
                        nc.sync.dma_start(
                            out=yts[e].ap()[:, 0:4 * C],
                            in_=yo[:, 0:4 * C])
                    elif m == 6:
                        # scalar (the other HWDGE) so this issue doesn't
                        # serialize ahead of m7's final DMA on sync.
                        nc.scalar.dma_start(
                            out=yts[e].ap()[:, 4 * C:7 * C],
                            in_=yo[:, 4 * C:7 * C])
                    elif m == MD - 1:
                        nc.sync.dma_start(
                            out=yts[e].ap()[:, m * C:(m + 1) * C],
                            in_=yo[:, m * C:(m + 1) * C])
    nc.compile()
    return nc


def _get_nc(reps: int = 1):
    if reps not in _CACHE:
        _CACHE[reps] = _build(reps)
    return _CACHE[reps]


def _route(gate_idx, gate_score):
    """Dedup routing + slot assignment. Returns per-expert
    (tokens, weights, overflow_tokens, overflow_weights, core, slot)."""
    g = np.asarray(gate_idx).astype(np.int64)
    sc = np.asarray(gate_score, dtype=np.float32)
    toks_all, wts_all, counts = [], [], []
    for e in range(E):
        m0, m1 = g[:, 0] == e, g[:, 1] == e
        toks = np.flatnonzero(m0 | m1)
        toks_all.append(toks)
        wts_all.append((sc[:, 0] * m0 + sc[:, 1] * m1)[toks])
        counts.append(len(toks))
    order = np.argsort(-np.asarray(counts), kind="stable")
    out = [None] * E
    for rank, e in enumerate(order):
        slot = 0 if rank < NCORES else 1
        core = rank if rank < NCORES else rank - NCORES
        cap = CAPS[slot]
        toks, wts = toks_all[e], wts_all[e]
        out[e] = (toks[:cap], wts[:cap], toks[cap:], wts[cap:], core, slot)
    return out


def kernel(inp, gate_idx, gate_score, w1, b1, w2, b2):
    inp = np.asarray(inp, dtype=np.float32)
    gate_idx = np.asarray(gate_idx)
    gate_score = np.asarray(gate_score, dtype=np.float32)
    w1 = np.asarray(w1, dtype=np.float32)
    b1 = np.asarray(b1, dtype=np.float32)
    w2 = np.asarray(w2, dtype=np.float32)
    b2 = np.asarray(b2, dtype=np.float32)

    routes = _route(gate_idx, gate_score)

    w1t_all = w1.transpose(0, 2, 1).astype(_F16)  # [E, D, H]
    w2t_all = np.ascontiguousarray(w2.transpose(0, 2, 1)).astype(_F16)
    b1r = np.ascontiguousarray(
        b1.reshape(E, KH, 128).transpose(0, 2, 1))  # [E, 128, KH]

    in_maps = [
        {"w1a": np.zeros((EPC, D, H // 2), _F16),
         "w1b": np.zeros((EPC, D, H // 2), _F16),
         "w2t": np.zeros((EPC, H, D), _F16),
         "b1": np.zeros((EPC, 128, KH), np.float32),
         "xt0": np.zeros((128, KD * CAPS[0]), _F16),
         "xt1": np.zeros((128, KD * CAPS[1]), _F16)}
        for _ in range(NCORES)
    ]
    for e in range(E):
        toks, wts, otoks, owts, core, slot = routes[e]
        im = in_maps[core]
        im["w1a"][slot] = w1t_all[e][:, :H // 2]
        im["w1b"][slot] = w1t_all[e][:, H // 2:]
        im["w2t"][slot] = w2t_all[e]
        im["b1"][slot] = b1r[e]
        if len(toks):
            # pack partition-major: xt[p, k*C + c] = inp[toks[c], k*128 + p]
            xv = im[f"xt{slot}"].reshape(128, KD, CAPS[slot])
            xv[:, :, :len(toks)] = (
                inp[toks].T.astype(_F16)
                .reshape(KD, 128, len(toks)).transpose(1, 0, 2))

    nc = _get_nc()
    res = run_bass_kernel_spmd(nc, in_maps, list(range(NCORES)))

    # Host combine: weight each expert's output columns by the (summed)
    # gate score and accumulate per token; add the b2 term (folded out of
    # the device kernel). Tokens are unique within an expert, so the
    # fancy-indexed += is safe.
    out = np.einsum("tk,tkd->td", gate_score,
                    b2[np.asarray(gate_idx).astype(np.int64)])
    out = np.ascontiguousarray(out, dtype=np.float32)
    for e in range(E):
        toks, wts, otoks, owts, core, slot = routes[e]
        if len(toks):
            cap = CAPS[slot]
            # unpack partition-major yt [128, MD*C] -> [D, C]
            y = (res.results[core][f"yt{slot}"]
                 .reshape(128, MD, cap).transpose(1, 0, 2)
                 .reshape(D, cap)[:, :len(toks)].T)
            out[toks] += wts[:, None] * y.astype(np.float32)
        if len(otoks):  # exact host fallback for capacity overflow
            hh = inp[otoks] @ w1[e].T + b1[e]
            hh = 0.5 * hh * (1.0 + np.tanh(
                np.sqrt(2.0 / np.pi) * (hh + 0.044715 * hh ** 3)))
            out[otoks] += owts[:, None] * (hh @ w2[e].T)
    return out
